# revision 6
# baseline (speedup 1.0000x reference)
"""DoubleStreamBlock (MMDiT-style) Trainium2 kernel, 8 NeuronCores.

Sharding: tensor-parallel over the 16 attention heads (2 heads/core);
qkv/fc1 column-sharded, out/fc2 row-sharded.  Two device launches with
host-side partial-sum reduction between them (on-device AllReduce of the
21MB activations would be slower than the compute itself):

  L1: qkv projection (+folded adaLN norm), RoPE+spatial modulation,
      attention (2 heads/core over the full txt+img sequence),
      row-sharded output projection -> per-core partial [HID, T].
  host: sum partials, attention residual, norm2 stats.
  L2: fc1 (+folded norm2) -> gelu -> fc2 row-sharded -> partials.
  host: sum partials, final residual.

Activations live transposed on device: [feature, token] so every matmul
is lhsT=W-tile (stationary), rhs=activation (moving), and attention
q/k/v arrive as [head_dim, token] with no transposes.

PSUM discipline: all pools are allocated once at kernel top and fit in
the 8 banks statically (phase-scoped PSUM pools deadlock the tile
scheduler via released-zone overlap deps).
"""

import sys
import os

for _p in ("/opt/trn_rl_repo",):
    if _p not in sys.path:
        sys.path.insert(0, _p)

import numpy as np
import ml_dtypes
from contextlib import ExitStack

import concourse.bass as bass
import concourse.tile as tile
from concourse import bacc, mybir
from concourse import bass_utils

BF16 = ml_dtypes.bfloat16
BF = mybir.dt.bfloat16
F32 = mybir.dt.float32
AF = mybir.ActivationFunctionType
ALU = mybir.AluOpType
AX = mybir.AxisListType

L, NI, T = 512, 2048, 2560          # txt tokens, img tokens, total
HID, NH, HD, MH = 2048, 16, 128, 8192
NC, HPC = 8, 2                       # cores, heads per core
SCALE = float(HD) ** -0.5
EPS = 1e-6
KC = HID // 128                      # 16 contraction chunks
NKT = T // 128                       # 20 key tiles


def _din(nc, name, shape, dt=BF):
    return nc.dram_tensor(name, shape, dt, kind="ExternalInput").ap()


def _dout(nc, name, shape, dt=F32):
    return nc.dram_tensor(name, shape, dt, kind="ExternalOutput").ap()


# --------------------------------------------------------------------------
# L1: qkv + rope/mod + attention + out-proj
# --------------------------------------------------------------------------

def build_l1():
    nc = bacc.Bacc("TRN2", target_bir_lowering=False, debug=False)
    A = {}
    A["xt_txt"] = _din(nc, "xt_txt", [HID, L])
    A["xt_img"] = _din(nc, "xt_img", [HID, NI])
    A["rs1"] = _din(nc, "rs1", [1, T], F32)
    A["wq_txt"] = _din(nc, "wq_txt", [HID, 768])
    A["wq_img"] = _din(nc, "wq_img", [HID, 768])
    A["qb_txt"] = _din(nc, "qb_txt", [128, 6], F32)
    A["qb_img"] = _din(nc, "qb_img", [128, 6], F32)
    A["cm"] = _din(nc, "cm", [HPC, 128, NI])
    A["sm"] = _din(nc, "sm", [HPC, 128, NI])
    A["permT"] = _din(nc, "permT", [128, 128])
    A["ident"] = _din(nc, "ident", [128, 128])
    A["identf"] = _din(nc, "identf", [128, 128], F32)
    A["s2"] = _din(nc, "s2", [128, 1], F32)
    A["ns2"] = _din(nc, "ns2", [128, 1], F32)
    A["wo_txt"] = _din(nc, "wo_txt", [HPC * HD, HID])
    A["wo_img"] = _din(nc, "wo_img", [HPC * HD, HID])
    A["part1"] = _dout(nc, "part1", [HID, T])

    with tile.TileContext(nc) as tc:
        _l1_body(tc, A)
    nc.compile()
    return nc


def _l1_body(tc, A):
    nc = tc.nc
    STAGE = int(os.environ.get('DBG_L1_STAGE', '9'))
    with ExitStack() as ctx:
        # ---- static psum pools: 2 + 5 + 1 = 8 banks.  The scores pool is
        # shared (same tag) with the P/V transposes: 5 slots so one full
        # q-tile row of score blocks can be live at once.
        pacc = ctx.enter_context(tc.tile_pool(name="pacc", bufs=2, space="PSUM"))
        psc = ctx.enter_context(tc.tile_pool(name="psc", bufs=5, space="PSUM"))
        prd = ctx.enter_context(tc.tile_pool(name="prd", bufs=1, space="PSUM"))

        const = ctx.enter_context(tc.tile_pool(name="const", bufs=1))
        qkvp = ctx.enter_context(tc.tile_pool(name="qkvp", bufs=1))
        attp = ctx.enter_context(tc.tile_pool(name="attp", bufs=1))

        ident_s = const.tile([128, 128], BF, tag="ident")
        nc.sync.dma_start(ident_s[:], A["ident"][:, :])
        identf_s = const.tile([128, 128], F32, tag="identf")
        nc.sync.dma_start(identf_s[:], A["identf"][:, :])
        permT_s = const.tile([128, 128], BF, tag="permT")
        nc.sync.dma_start(permT_s[:], A["permT"][:, :])
        s2_s = const.tile([128, 1], F32, tag="s2")
        nc.sync.dma_start(s2_s[:], A["s2"][:, :])
        ns2_s = const.tile([128, 1], F32, tag="ns2")
        nc.sync.dma_start(ns2_s[:], A["ns2"][:, :])
        qb_s = []
        for s in (0, 1):
            t = const.tile([128, 6], F32, tag=f"qb{s}")
            nc.sync.dma_start(t[:], A["qb_txt" if s == 0 else "qb_img"][:, :])
            qb_s.append(t)
        rs1_row = const.tile([1, T], F32, tag="rs1row")
        nc.sync.dma_start(rs1_row[:], A["rs1"][:, :])
        rs1b = const.tile([128, T], F32, tag="rs1b")
        nc.gpsimd.partition_broadcast(rs1b[:], rs1_row[0:1, :])

        # persistent q/k/v and attention-out tiles (bf16, [128, tokens])
        q_txt = [qkvp.tile([128, L], BF, tag=f"qtxt{h}", name=f"qtxt{h}")
                 for h in range(HPC)]
        q_imgr = [qkvp.tile([128, NI], BF, tag=f"qimgr{h}", name=f"qimgr{h}")
                  for h in range(HPC)]
        q_img = [qkvp.tile([128, NI], BF, tag=f"qimg{h}", name=f"qimg{h}")
                 for h in range(HPC)]
        k_imgr = [qkvp.tile([128, NI], BF, tag=f"kimgr{h}", name=f"kimgr{h}")
                  for h in range(HPC)]
        kcat = [qkvp.tile([128, T], BF, tag=f"kcat{h}", name=f"kcat{h}")
                for h in range(HPC)]
        vcat = [qkvp.tile([128, T], BF, tag=f"vcat{h}", name=f"vcat{h}")
                for h in range(HPC)]
        att = [attp.tile([128, T], BF, tag=f"att{h}", name=f"att{h}")
               for h in range(HPC)]

        # ------------------- phase 1: qkv projection -------------------
        with ExitStack() as c1:
            xp = c1.enter_context(tc.tile_pool(name="xt1", bufs=32))
            wp = c1.enter_context(tc.tile_pool(name="wb1", bufs=16))
            tp = c1.enter_context(tc.tile_pool(name="tmp_qkv", bufs=3))

            for s in (0, 1):
                Tn = L if s == 0 else NI
                col0 = 0 if s == 0 else L
                xt_d = A["xt_txt"] if s == 0 else A["xt_img"]
                wq_d = A["wq_txt"] if s == 0 else A["wq_img"]

                wbs = []
                for kc in range(KC):
                    wb = wp.tile([128, 768], BF, tag="wb")
                    nc.sync.dma_start(wb[:], wq_d[kc * 128:(kc + 1) * 128, :])
                    wbs.append(wb)

                for ci in range(Tn // 512):
                    cs = slice(ci * 512, (ci + 1) * 512)
                    xts = []
                    for kc in range(KC):
                        xtile = xp.tile([128, 512], BF, tag="xch")
                        nc.sync.dma_start(xtile[:],
                                          xt_d[kc * 128:(kc + 1) * 128, cs])
                        xts.append(xtile)
                    for m in range(6):
                        ps_t = pacc.tile([128, 512], F32, tag="acc")
                        for kc in range(KC):
                            nc.tensor.matmul(ps_t[:],
                                             wbs[kc][:, m * 128:(m + 1) * 128],
                                             xts[kc][:],
                                             start=(kc == 0), stop=(kc == KC - 1))
                        tmp = tp.tile([128, 512], BF, tag="evtmp")
                        nc.vector.tensor_tensor(
                            tmp[:], ps_t[:],
                            rs1b[:, col0 + ci * 512:col0 + (ci + 1) * 512],
                            ALU.mult)
                        h = m % 2
                        kind = m // 2  # 0=q 1=k 2=v
                        if kind == 0:
                            dtile = q_txt[h] if s == 0 else q_imgr[h]
                            dst = dtile[:, cs]
                        elif kind == 1:
                            dst = kcat[h][:, cs] if s == 0 else k_imgr[h][:, cs]
                        else:
                            dst = vcat[h][:, cs] if s == 0 \
                                else vcat[h][:, L + ci * 512:L + (ci + 1) * 512]
                        nc.scalar.add(dst, tmp[:], qb_s[s][:, m:m + 1])

        # ------------------- phase 2: rope + modulation ----------------
        with ExitStack() as c2:
            if STAGE < 2:
                return
            cmp_ = c2.enter_context(tc.tile_pool(name="cmsm", bufs=2))
            rtt = c2.enter_context(tc.tile_pool(name="tmp_rope", bufs=4))

            for h in range(HPC):
                cm_s = cmp_.tile([128, NI], BF, tag="cm")
                nc.sync.dma_start(cm_s[:], A["cm"][h])
                sm_s = cmp_.tile([128, NI], BF, tag="sm")
                nc.sync.dma_start(sm_s[:], A["sm"][h])
                for src, dsttile, d0 in ((q_imgr[h], q_img[h], 0),
                                         (k_imgr[h], kcat[h], L)):
                    for ci in range(NI // 512):
                        cs = slice(ci * 512, (ci + 1) * 512)
                        ds = slice(d0 + ci * 512, d0 + (ci + 1) * 512)
                        rps = pacc.tile([128, 512], F32, tag="acc")
                        nc.tensor.matmul(rps[:], permT_s[:], src[:, cs],
                                         start=True, stop=True)
                        t1 = rtt.tile([128, 512], BF, tag="t1")
                        nc.vector.tensor_tensor(t1[:], src[:, cs], cm_s[:, cs],
                                                ALU.mult)
                        t2 = rtt.tile([128, 512], BF, tag="t2")
                        nc.vector.tensor_tensor(t2[:], rps[:], sm_s[:, cs],
                                                ALU.mult)
                        nc.vector.tensor_tensor(dsttile[:, ds], t1[:], t2[:],
                                                ALU.add)

        # ------------------- phase 3: attention ------------------------
        with ExitStack() as c3:
            if STAGE < 3:
                return
            vtp = c3.enter_context(tc.tile_pool(name="vtp", bufs=1))
            ptp = c3.enter_context(tc.tile_pool(name="ptp", bufs=2))
            prp = c3.enter_context(tc.tile_pool(name="prp", bufs=2))
            dnp = c3.enter_context(tc.tile_pool(name="dnp", bufs=3))

            vts = []
            for h in range(HPC):
                vt = vtp.tile([128, NKT, 128], BF, tag=f"vt{h}", name=f"vt{h}")
                for j in range(NKT):
                    tp_ps = psc.tile([128, 128], BF, tag="scps")
                    nc.tensor.transpose(tp_ps[:],
                                        vcat[h][:, j * 128:(j + 1) * 128],
                                        ident_s[:])
                    nc.scalar.copy(vt[:, j, :], tp_ps[:])
                vts.append(vt)

            if STAGE < 4:
                return
            for s in (0, 1):
                Tq = L if s == 0 else NI
                ocol0 = 0 if s == 0 else L
                for h in range(HPC):
                    q_t = q_txt[h] if s == 0 else q_img[h]
                    for g in range(Tq // 512):
                        pt = ptp.tile([128, NKT, 512], BF, tag="pt")
                        denrow = dnp.tile([1, 512], F32, tag="denrow")
                        for qt in range(4):
                            qc = g * 512 + qt * 128
                            pr_t = prp.tile([128, T], BF, tag="pr")
                            mx5 = dnp.tile([128, 5], F32, tag="mx5")
                            scps_list = []
                            for b in range(5):
                                scp = psc.tile([128, 512], F32, tag="scps")
                                nc.tensor.matmul(
                                    scp[:], q_t[:, qc:qc + 128],
                                    kcat[h][:, b * 512:(b + 1) * 512],
                                    start=True, stop=True)
                                nc.vector.tensor_reduce(
                                    mx5[:, b:b + 1], scp[:],
                                    axis=AX.X, op=ALU.max)
                                scps_list.append(scp)
                            if STAGE < 5:
                                continue
                            mx = dnp.tile([128, 1], F32, tag="mx")
                            nc.vector.tensor_reduce(mx[:], mx5[:, 0:5],
                                                    axis=AX.X, op=ALU.max)
                            nb = dnp.tile([128, 1], F32, tag="nb")
                            den5 = dnp.tile([128, 5], F32, tag="den5")
                            if s == 0:
                                nc.vector.tensor_scalar_mul(nb[:], mx[:], -SCALE)
                            else:
                                nc.vector.tensor_tensor(nb[:], mx[:], ns2_s[:],
                                                        ALU.mult)
                            for b in range(5):
                                nc.scalar.activation(
                                    pr_t[:, b * 512:(b + 1) * 512],
                                    scps_list[b][:],
                                    AF.Exp, bias=nb[:],
                                    scale=(SCALE if s == 0 else s2_s[:]),
                                    accum_out=den5[:, b:b + 1])
                            if STAGE < 6:
                                continue
                            den = dnp.tile([128, 1], F32, tag="den")
                            nc.vector.tensor_reduce(den[:], den5[:, 0:5],
                                                    axis=AX.X, op=ALU.add)
                            rden = dnp.tile([128, 1], F32, tag="rden")
                            nc.vector.reciprocal(rden[:], den[:])
                            rd_ps = prd.tile([1, 128], F32, tag="rdps")
                            nc.tensor.transpose(rd_ps[:], rden[:], identf_s[:])
                            nc.scalar.copy(denrow[0:1, qt * 128:(qt + 1) * 128],
                                           rd_ps[0:1, :])
                            if STAGE < 7:
                                continue
                            for j in range(NKT):
                                tp_ps = psc.tile([128, 128], BF, tag="scps")
                                nc.tensor.transpose(
                                    tp_ps[:], pr_t[:, j * 128:(j + 1) * 128],
                                    ident_s[:])
                                dst = pt[:, j, qt * 128:(qt + 1) * 128]
                                if j % 2 == 0:
                                    nc.scalar.copy(dst, tp_ps[:])
                                else:
                                    nc.vector.tensor_copy(dst, tp_ps[:])
                        if STAGE < 8:
                            continue
                        rdenb = dnp.tile([128, 512], F32, tag="rdenb")
                        nc.gpsimd.partition_broadcast(rdenb[:], denrow[0:1, :])
                        pv_ps = pacc.tile([128, 512], F32, tag="acc")
                        for j in range(NKT):
                            nc.tensor.matmul(pv_ps[:], vts[h][:, j, :],
                                             pt[:, j, :],
                                             start=(j == 0), stop=(j == NKT - 1))
                        oc = slice(ocol0 + g * 512, ocol0 + (g + 1) * 512)
                        nc.vector.tensor_tensor(att[h][:, oc], pv_ps[:],
                                                rdenb[:], ALU.mult)

        # ------------------- phase 4: out projection -------------------
        with ExitStack() as c4:
            if STAGE < 9:
                return
            wop = c4.enter_context(tc.tile_pool(name="wop", bufs=4))
            oev = c4.enter_context(tc.tile_pool(name="oev", bufs=4))

            for s in (0, 1):
                Tn = L if s == 0 else NI
                col0 = 0 if s == 0 else L
                wo_d = A["wo_txt"] if s == 0 else A["wo_img"]
                wob = []
                for kc in range(HPC):
                    wb = wop.tile([128, HID], BF, tag="wob")
                    nc.sync.dma_start(wb[:], wo_d[kc * 128:(kc + 1) * 128, :])
                    wob.append(wb)
                for m in range(KC):
                    for ci in range(Tn // 512):
                        acs = slice(col0 + ci * 512, col0 + (ci + 1) * 512)
                        ps_o = pacc.tile([128, 512], F32, tag="acc")
                        for kc in range(HPC):
                            nc.tensor.matmul(ps_o[:],
                                             wob[kc][:, m * 128:(m + 1) * 128],
                                             att[kc][:, acs],
                                             start=(kc == 0),
                                             stop=(kc == HPC - 1))
                        ev = oev.tile([128, 512], F32, tag="oevt")
                        if ci % 2 == 0:
                            nc.vector.tensor_copy(ev[:], ps_o[:])
                        else:
                            nc.scalar.copy(ev[:], ps_o[:])
                        nc.sync.dma_start(
                            A["part1"][m * 128:(m + 1) * 128, acs], ev[:])


# --------------------------------------------------------------------------
# L2: fc1 + gelu + fc2
# --------------------------------------------------------------------------

def build_l2():
    nc = bacc.Bacc("TRN2", target_bir_lowering=False, debug=False)
    A = {}
    A["x2t"] = _din(nc, "x2t", [HID, T])
    A["rs2"] = _din(nc, "rs2", [1, T], F32)
    A["w1_txt"] = _din(nc, "w1_txt", [HID, MH // NC])
    A["w1_img"] = _din(nc, "w1_img", [HID, MH // NC])
    A["b1_txt"] = _din(nc, "b1_txt", [128, 8], F32)
    A["b1_img"] = _din(nc, "b1_img", [128, 8], F32)
    A["w2_txt"] = _din(nc, "w2_txt", [MH // NC, HID])
    A["w2_img"] = _din(nc, "w2_img", [MH // NC, HID])
    A["part2"] = _dout(nc, "part2", [HID, T])

    with tile.TileContext(nc) as tc:
        _l2_body(tc, A)
    nc.compile()
    return nc


def _l2_body(tc, A):
    nc = tc.nc
    MS = MH // NC            # 1024 mlp cols per core
    M1 = MS // 128           # 8 fc1 m-tiles
    with ExitStack() as ctx:
        # static psum: 4 + 4 = 8 banks
        pf1 = ctx.enter_context(tc.tile_pool(name="pf1", bufs=4, space="PSUM"))
        pf2 = ctx.enter_context(tc.tile_pool(name="pf2", bufs=4, space="PSUM"))

        cp = ctx.enter_context(tc.tile_pool(name="const2", bufs=1))
        hp = ctx.enter_context(tc.tile_pool(name="hp", bufs=M1))
        xp = ctx.enter_context(tc.tile_pool(name="xt2", bufs=KC))
        w1p = ctx.enter_context(tc.tile_pool(name="w1b", bufs=KC))
        w2p = ctx.enter_context(tc.tile_pool(name="w2b", bufs=M1))
        tp = ctx.enter_context(tc.tile_pool(name="tmp2", bufs=3))
        oev = ctx.enter_context(tc.tile_pool(name="oev2", bufs=4))

        rs2_row = cp.tile([1, T], F32, tag="rs2row")
        nc.sync.dma_start(rs2_row[:], A["rs2"][:, :])
        rs2b = cp.tile([128, T], F32, tag="rs2b")
        nc.gpsimd.partition_broadcast(rs2b[:], rs2_row[0:1, :])
        b1_s = []
        for s in (0, 1):
            t = cp.tile([128, M1], F32, tag=f"b1{s}")
            nc.sync.dma_start(t[:], A["b1_txt" if s == 0 else "b1_img"][:, :])
            b1_s.append(t)

        for s in (0, 1):
            Tn = L if s == 0 else NI
            col0 = 0 if s == 0 else L
            nch = Tn // 512
            w1_d = A["w1_txt"] if s == 0 else A["w1_img"]
            w2_d = A["w2_txt"] if s == 0 else A["w2_img"]

            xts = []
            for kc in range(KC):
                xtile = xp.tile([128, NI], BF, tag="x2")
                nc.sync.dma_start(xtile[:, 0:Tn],
                                  A["x2t"][kc * 128:(kc + 1) * 128,
                                           col0:col0 + Tn])
                xts.append(xtile)
            w1bs = []
            for kc in range(KC):
                wb = w1p.tile([128, MS], BF, tag="w1")
                nc.sync.dma_start(wb[:], w1_d[kc * 128:(kc + 1) * 128, :])
                w1bs.append(wb)

            hts = []
            for m in range(M1):
                pss = [pf1.tile([128, 512], F32, tag="f1", name=f"f1_{s}_{m}_{i}") for i in range(nch)]
                for kc in range(KC):
                    for ci in range(nch):
                        cs = slice(ci * 512, (ci + 1) * 512)
                        nc.tensor.matmul(pss[ci][:],
                                         w1bs[kc][:, m * 128:(m + 1) * 128],
                                         xts[kc][:, cs],
                                         start=(kc == 0), stop=(kc == KC - 1))
                ht = hp.tile([128, NI], BF, tag="h")
                for ci in range(nch):
                    cs = slice(ci * 512, (ci + 1) * 512)
                    tmp = tp.tile([128, 512], BF, tag="f1tmp")
                    nc.vector.tensor_tensor(
                        tmp[:], pss[ci][:],
                        rs2b[:, col0 + ci * 512:col0 + (ci + 1) * 512],
                        ALU.mult)
                    nc.scalar.activation(ht[:, cs], tmp[:],
                                         AF.Gelu_apprx_tanh,
                                         bias=b1_s[s][:, m:m + 1], scale=1.0)
                hts.append(ht)

            w2bs = []
            for kc in range(M1):
                wb = w2p.tile([128, HID], BF, tag="w2")
                nc.sync.dma_start(wb[:], w2_d[kc * 128:(kc + 1) * 128, :])
                w2bs.append(wb)

            for m2 in range(KC):
                pss = [pf2.tile([128, 512], F32, tag="f2", name=f"f2_{s}_{m2}_{i}") for i in range(nch)]
                for kc in range(M1):
                    for ci in range(nch):
                        cs = slice(ci * 512, (ci + 1) * 512)
                        nc.tensor.matmul(pss[ci][:],
                                         w2bs[kc][:, m2 * 128:(m2 + 1) * 128],
                                         hts[kc][:, cs],
                                         start=(kc == 0), stop=(kc == M1 - 1))
                for ci in range(nch):
                    gcs = slice(col0 + ci * 512, col0 + (ci + 1) * 512)
                    ev = oev.tile([128, 512], F32, tag="oevt2")
                    if ci % 2 == 0:
                        nc.vector.tensor_copy(ev[:], pss[ci][:])
                    else:
                        nc.scalar.copy(ev[:], pss[ci][:])
                    nc.sync.dma_start(A["part2"][m2 * 128:(m2 + 1) * 128, gcs],
                                      ev[:])


# --------------------------------------------------------------------------
# host math
# --------------------------------------------------------------------------

def _silu(x):
    return x / (1.0 + np.exp(-x))


def _resize_bilinear_8x8(a, H, W):
    """jax.image.resize(..., 'bilinear') equivalent (half-pixel centers)."""
    ih, iw = a.shape
    y = (np.arange(H, dtype=np.float64) + 0.5) * (ih / H) - 0.5
    x = (np.arange(W, dtype=np.float64) + 0.5) * (iw / W) - 0.5
    y0 = np.floor(y).astype(np.int64)
    x0 = np.floor(x).astype(np.int64)
    wy = (y - y0).astype(np.float32)
    wx = (x - x0).astype(np.float32)
    y0c = np.clip(y0, 0, ih - 1)
    y1c = np.clip(y0 + 1, 0, ih - 1)
    x0c = np.clip(x0, 0, iw - 1)
    x1c = np.clip(x0 + 1, 0, iw - 1)
    a = a.astype(np.float32)
    top = a[y0c][:, x0c] * (1 - wx)[None, :] + a[y0c][:, x1c] * wx[None, :]
    bot = a[y1c][:, x0c] * (1 - wx)[None, :] + a[y1c][:, x1c] * wx[None, :]
    return top * (1 - wy)[:, None] + bot * wy[:, None]


_PROGS = {}


def _get_progs():
    if "l1" not in _PROGS:
        _PROGS["l1"] = build_l1()
        _PROGS["l2"] = build_l2()
    return _PROGS["l1"], _PROGS["l2"]


def _run(nc, in_maps, trace=False):
    return bass_utils.run_bass_kernel_spmd(
        nc, in_maps, core_ids=list(range(NC)), trace=trace)


def kernel(txt, img, vec, rope, sol_temperature, sol_spatial,
           ada_img_w, ada_img_b, ada_img_nw, ada_txt_w, ada_txt_b, ada_txt_nw,
           txt_qkv_w, img_qkv_w, txt_out_w, img_out_w, mod_w, mod_b,
           img_n2_w, txt_n2_w, img_fc1_w, img_fc1_b, img_fc2_w, img_fc2_b,
           txt_fc1_w, txt_fc1_b, txt_fc2_w, txt_fc2_b, spatial_h, spatial_w,
           _trace=False, _timing=None):
    f = np.float32
    nc1, nc2 = _get_progs()

    x_txt = np.asarray(txt, f)[0]            # [L, HID]
    x_img = np.asarray(img, f)[0]            # [NI, HID]
    v = np.asarray(vec, f)[0]                # [HID]

    sv = _silu(v)
    e_txt = sv @ np.asarray(ada_txt_w, f) + np.asarray(ada_txt_b, f)
    e_img = sv @ np.asarray(ada_img_w, f) + np.asarray(ada_img_b, f)
    shm_t, scm_t, gm_t, shp_t, scp_t, gp_t = np.split(e_txt, 6)
    shm_i, scm_i, gm_i, shp_i, scp_i, gp_i = np.split(e_img, 6)

    a1_t = np.asarray(ada_txt_nw, f) * (1.0 + scm_t)
    a1_i = np.asarray(ada_img_nw, f) * (1.0 + scm_i)

    rs1_t = 1.0 / np.sqrt((x_txt * x_txt).mean(-1) + EPS)     # [L]
    rs1_i = 1.0 / np.sqrt((x_img * x_img).mean(-1) + EPS)     # [NI]
    rs1 = np.concatenate([rs1_t, rs1_i]).astype(f)[None, :]   # [1, T]

    # spatial modulation folded into rope cos/sin
    up = _resize_bilinear_8x8(np.asarray(sol_spatial, f)[0], 64, 32).reshape(-1)
    mod = np.exp(np.clip(up[None, :] * np.asarray(mod_w, f)[:, None]
                         + np.asarray(mod_b, f)[:, None], -2.0, 2.0))  # [NH, NI]
    fr = np.asarray(rope, f)[:, 0]                        # [NI, HD]
    cos = np.repeat(fr[:, 0::2], 2, axis=1)               # [NI, HD]
    sin = np.repeat(fr[:, 1::2], 2, axis=1)

    temp = max(float(np.asarray(sol_temperature, f).mean()), 0.1)
    s2v = SCALE / temp
    s2 = np.full((128, 1), s2v, f)
    ns2 = -s2

    permT = np.zeros((128, 128), f)
    for i in range(64):
        permT[2 * i + 1, 2 * i] = -1.0
        permT[2 * i, 2 * i + 1] = 1.0
    ident = np.eye(128, dtype=f)

    wq_t_full = np.asarray(txt_qkv_w, f)
    wq_i_full = np.asarray(img_qkv_w, f)
    wo_t_full = np.asarray(txt_out_w, f)
    wo_i_full = np.asarray(img_out_w, f)

    xt_txt = np.ascontiguousarray(x_txt.T).astype(BF16)
    xt_img = np.ascontiguousarray(x_img.T).astype(BF16)

    in_maps1 = []
    for c in range(NC):
        h0, h1 = 2 * c, 2 * c + 1
        cols = np.concatenate([
            np.arange(h0 * HD, (h0 + 1) * HD),
            np.arange(h1 * HD, (h1 + 1) * HD),
            HID + np.arange(h0 * HD, (h0 + 1) * HD),
            HID + np.arange(h1 * HD, (h1 + 1) * HD),
            2 * HID + np.arange(h0 * HD, (h0 + 1) * HD),
            2 * HID + np.arange(h1 * HD, (h1 + 1) * HD)])
        wq_t_raw = wq_t_full[:, cols]
        wq_i_raw = wq_i_full[:, cols]
        cm = np.stack([(cos * mod[h][:, None]).T for h in (h0, h1)]).astype(BF16)
        sm = np.stack([(sin * mod[h][:, None]).T for h in (h0, h1)]).astype(BF16)
        in_maps1.append({
            "xt_txt": xt_txt, "xt_img": xt_img, "rs1": rs1,
            "wq_txt": (a1_t[:, None] * wq_t_raw).astype(BF16),
            "wq_img": (a1_i[:, None] * wq_i_raw).astype(BF16),
            "qb_txt": np.ascontiguousarray(
                (shm_t @ wq_t_raw).astype(f).reshape(6, 128).T),
            "qb_img": np.ascontiguousarray(
                (shm_i @ wq_i_raw).astype(f).reshape(6, 128).T),
            "cm": cm, "sm": sm,
            "permT": permT.astype(BF16), "ident": ident.astype(BF16),
            "identf": ident, "s2": s2, "ns2": ns2,
            "wo_txt": wo_t_full[c * 256:(c + 1) * 256, :].astype(BF16),
            "wo_img": wo_i_full[c * 256:(c + 1) * 256, :].astype(BF16),
        })

    res1 = _run(nc1, in_maps1, trace=_trace)
    if _timing is not None:
        _timing.append(("l1", res1.exec_time_ns))

    attnT = res1.results[0]["part1"].astype(f)
    for c in range(1, NC):
        attnT += res1.results[c]["part1"]

    x2_t = x_txt.T + gm_t[:, None] * attnT[:, :L]     # [HID, L]
    x2_i = x_img.T + gm_i[:, None] * attnT[:, L:]     # [HID, NI]

    rs2_t = 1.0 / np.sqrt((x2_t * x2_t).mean(0) + EPS)
    rs2_i = 1.0 / np.sqrt((x2_i * x2_i).mean(0) + EPS)
    rs2 = np.concatenate([rs2_t, rs2_i]).astype(f)[None, :]

    x2t = np.concatenate([x2_t, x2_i], axis=1).astype(BF16)

    a2_t = np.asarray(txt_n2_w, f) * (1.0 + scp_t)
    a2_i = np.asarray(img_n2_w, f) * (1.0 + scp_i)
    b1f_t = shp_t @ np.asarray(txt_fc1_w, f) + np.asarray(txt_fc1_b, f)
    b1f_i = shp_i @ np.asarray(img_fc1_w, f) + np.asarray(img_fc1_b, f)

    MS = MH // NC
    in_maps2 = []
    for c in range(NC):
        csl = slice(c * MS, (c + 1) * MS)
        in_maps2.append({
            "x2t": x2t, "rs2": rs2,
            "w1_txt": (a2_t[:, None]
                       * np.asarray(txt_fc1_w, f)[:, csl]).astype(BF16),
            "w1_img": (a2_i[:, None]
                       * np.asarray(img_fc1_w, f)[:, csl]).astype(BF16),
            "b1_txt": np.ascontiguousarray(b1f_t[csl].reshape(8, 128).T.astype(f)),
            "b1_img": np.ascontiguousarray(b1f_i[csl].reshape(8, 128).T.astype(f)),
            "w2_txt": np.asarray(txt_fc2_w, f)[csl, :].astype(BF16),
            "w2_img": np.asarray(img_fc2_w, f)[csl, :].astype(BF16),
        })

    res2 = _run(nc2, in_maps2, trace=_trace)
    if _timing is not None:
        _timing.append(("l2", res2.exec_time_ns))

    mlpT = res2.results[0]["part2"].astype(f)
    for c in range(1, NC):
        mlpT += res2.results[c]["part2"]

    out_t = x2_t + gp_t[:, None] * (mlpT[:, :L]
                                    + np.asarray(txt_fc2_b, f)[:, None])
    out_i = x2_i + gp_i[:, None] * (mlpT[:, L:]
                                    + np.asarray(img_fc2_b, f)[:, None])

    txt_out = np.ascontiguousarray(out_t.T).reshape(1, L, HID).astype(f)
    img_out = np.ascontiguousarray(out_i.T).reshape(1, NI, HID).astype(f)
    return txt_out, img_out


# revision 19
# speedup vs baseline: 1.2653x; 1.2653x over previous
"""DoubleStreamBlock (MMDiT-style) Trainium2 kernel, 8 NeuronCores.

Sharding: tensor-parallel over the 16 attention heads (2 heads/core);
qkv/fc1 column-sharded, out/fc2 row-sharded.  Two device launches with
host-side partial-sum reduction between them (on-device AllReduce of the
21MB activations would be slower than the compute itself):

  L1: qkv projection (+folded adaLN norm), RoPE+spatial modulation,
      attention (2 heads/core over the full txt+img sequence),
      row-sharded output projection -> per-core partial [HID, T].
  host: sum partials, attention residual, norm2 stats.
  L2: fc1 (+folded norm2) -> gelu -> fc2 row-sharded -> partials.
  host: sum partials, final residual.

Activations live transposed on device: [feature, token] so every matmul
is lhsT=W-tile (stationary), rhs=activation (moving), and attention
q/k/v arrive as [head_dim, token] with no transposes.

PSUM discipline: all pools are allocated once at kernel top and fit in
the 8 banks statically (phase-scoped PSUM pools deadlock the tile
scheduler via released-zone overlap deps).
"""

import sys
import os

for _p in ("/opt/trn_rl_repo",):
    if _p not in sys.path:
        sys.path.insert(0, _p)

import numpy as np
import ml_dtypes
from contextlib import ExitStack

import concourse.bass as bass
import concourse.tile as tile
from concourse import bacc, mybir
from concourse import bass_utils

BF16 = ml_dtypes.bfloat16
BF = mybir.dt.bfloat16
F32 = mybir.dt.float32
AF = mybir.ActivationFunctionType
ALU = mybir.AluOpType
AX = mybir.AxisListType

L, NI, T = 512, 2048, 2560          # txt tokens, img tokens, total
HID, NH, HD, MH = 2048, 16, 128, 8192
NC, HPC = 8, 2                       # cores, heads per core
SCALE = float(HD) ** -0.5
EPS = 1e-6
KC = HID // 128                      # 16 contraction chunks
NKT = T // 128                       # 20 key tiles


def _din(nc, name, shape, dt=BF):
    return nc.dram_tensor(name, shape, dt, kind="ExternalInput").ap()


def _dout(nc, name, shape, dt=F32):
    return nc.dram_tensor(name, shape, dt, kind="ExternalOutput").ap()


# --------------------------------------------------------------------------
# L1: qkv + rope/mod + attention + out-proj
# --------------------------------------------------------------------------

def build_l1():
    nc = bacc.Bacc("TRN2", target_bir_lowering=False, debug=False)
    A = {}
    A["xt_txt"] = _din(nc, "xt_txt", [HID, L])
    A["xt_img"] = _din(nc, "xt_img", [HID, NI])
    A["wq_txt"] = _din(nc, "wq_txt", [HID, 768])
    A["wq_img"] = _din(nc, "wq_img", [HID, 768])
    A["qb_txt"] = _din(nc, "qb_txt", [128, 6], F32)
    A["qb_img"] = _din(nc, "qb_img", [128, 6], F32)
    A["cm"] = _din(nc, "cm", [HPC, 128, NI])
    A["sm"] = _din(nc, "sm", [HPC, 128, NI])
    A["permT"] = _din(nc, "permT", [128, 128])
    A["ident"] = _din(nc, "ident", [128, 128])
    A["identf"] = _din(nc, "identf", [128, 128], F32)
    A["s2"] = _din(nc, "s2", [128, 1], F32)
    A["ns2"] = _din(nc, "ns2", [128, 1], F32)
    A["wo_txt"] = _din(nc, "wo_txt", [HPC * HD, HID])
    A["wo_img"] = _din(nc, "wo_img", [HPC * HD, HID])
    A["part1"] = _dout(nc, "part1", [HID, T])

    with tile.TileContext(nc) as tc:
        _l1_body(tc, A)
    nc.compile()
    return nc


def _l1_body(tc, A):
    nc = tc.nc
    with ExitStack() as ctx:
        # ---- static psum pools: 1 + 4 + 2 + 1 = 8 banks.  Scores use
        # per-block softmax (block max + correction factors) so score psum
        # frees right after its block exp; transposes get their own pool.
        pacc = ctx.enter_context(tc.tile_pool(name="pacc", bufs=1, space="PSUM"))
        psc = ctx.enter_context(tc.tile_pool(name="psc", bufs=4, space="PSUM"))
        ptt = ctx.enter_context(tc.tile_pool(name="ptt", bufs=2, space="PSUM"))
        prd = ctx.enter_context(tc.tile_pool(name="prd", bufs=1, space="PSUM"))

        const = ctx.enter_context(tc.tile_pool(name="const", bufs=1))
        qkvp = ctx.enter_context(tc.tile_pool(name="qkvp", bufs=1))
        attp = ctx.enter_context(tc.tile_pool(name="attp", bufs=1))

        ident_s = const.tile([128, 128], BF, tag="ident")
        identf_s = const.tile([128, 128], F32, tag="identf")
        permT_s = const.tile([128, 128], BF, tag="permT")
        s2_s = const.tile([128, 1], F32, tag="s2")
        ns2_s = const.tile([128, 1], F32, tag="ns2")
        qb_s = []
        for s in (0, 1):
            t = const.tile([128, 6], F32, tag=f"qb{s}")
            nc.sync.dma_start(t[:], A["qb_txt" if s == 0 else "qb_img"][:, :])
            qb_s.append(t)

        # persistent q/k/v and attention-out tiles (bf16, [128, tokens])
        q_txt = [qkvp.tile([128, L], BF, tag=f"qtxt{h}", name=f"qtxt{h}")
                 for h in range(HPC)]
        q_imgr = [qkvp.tile([128, NI], BF, tag=f"qimgr{h}", name=f"qimgr{h}")
                  for h in range(HPC)]
        q_img = [qkvp.tile([128, NI], BF, tag=f"qimg{h}", name=f"qimg{h}")
                 for h in range(HPC)]
        k_imgr = [qkvp.tile([128, NI], BF, tag=f"kimgr{h}", name=f"kimgr{h}")
                  for h in range(HPC)]
        kcat = [qkvp.tile([128, T], BF, tag=f"kcat{h}", name=f"kcat{h}")
                for h in range(HPC)]
        vcat = [qkvp.tile([128, T], BF, tag=f"vcat{h}", name=f"vcat{h}")
                for h in range(HPC)]
        att = [attp.tile([128, T], BF, tag=f"att{h}", name=f"att{h}")
               for h in range(HPC)]

        # ------------------- phase 1: qkv projection -------------------
        # img stream m-order computes q then k per head so RoPE overlaps the
        # remaining qkv matmuls; v comes last.
        with ExitStack() as c1:
            xp = c1.enter_context(tc.tile_pool(name="xt1", bufs=16))
            wp = c1.enter_context(tc.tile_pool(name="wb1", bufs=16))
            cmp_ = c1.enter_context(tc.tile_pool(name="cmsm", bufs=1))
            rtt = c1.enter_context(tc.tile_pool(name="tmp_rope", bufs=4))

            def rope_head(h, cm_s, sm_s):
                for src_t, dsttile, d0 in ((q_imgr[h], q_img[h], 0),
                                           (k_imgr[h], kcat[h], L)):
                    for ci in range(NI // 512):
                        cs = slice(ci * 512, (ci + 1) * 512)
                        ds = slice(d0 + ci * 512, d0 + (ci + 1) * 512)
                        rps = pacc.tile([128, 512], F32, tag="acc")
                        nc.tensor.matmul(rps[:], permT_s[:], src_t[:, cs],
                                         start=True, stop=True)
                        t1 = rtt.tile([128, 512], BF, tag="t1")
                        nc.vector.tensor_tensor(t1[:], src_t[:, cs],
                                                cm_s[:, cs], ALU.mult)
                        t2 = rtt.tile([128, 512], BF, tag="t2")
                        nc.vector.tensor_tensor(t2[:], rps[:], sm_s[:, cs],
                                                ALU.mult)
                        nc.vector.tensor_tensor(dsttile[:, ds], t1[:], t2[:],
                                                ALU.add)

            for s in (0, 1):
                Tn = L if s == 0 else NI
                col0 = 0 if s == 0 else L
                nch = Tn // 512
                xt_d = A["xt_txt"] if s == 0 else A["xt_img"]
                wq_d = A["wq_txt"] if s == 0 else A["wq_img"]

                wbs, xts = [], []
                for kc in range(KC):
                    wb = wp.tile([128, 768], BF, tag="wb")
                    nc.sync.dma_start(wb[:], wq_d[kc * 128:(kc + 1) * 128, :])
                    wbs.append(wb)
                    xtile = xp.tile([128, NI], BF, tag="xch")
                    nc.sync.dma_start(xtile[:, 0:Tn],
                                      xt_d[kc * 128:(kc + 1) * 128, :])
                    xts.append(xtile)
                if s == 0:
                    # consts ride behind the first stream's operands
                    nc.sync.dma_start(ident_s[:], A["ident"][:, :])
                    nc.sync.dma_start(identf_s[:], A["identf"][:, :])
                    nc.sync.dma_start(permT_s[:], A["permT"][:, :])
                    nc.sync.dma_start(s2_s[:], A["s2"][:, :])
                    nc.sync.dma_start(ns2_s[:], A["ns2"][:, :])
                    morder = range(6)
                    cm_ts = None
                else:
                    cm_ts = []
                    for h in range(HPC):
                        cm_s = cmp_.tile([128, NI], BF, tag=f"cm{h}",
                                         name=f"cm{h}")
                        nc.sync.dma_start(cm_s[:], A["cm"][h])
                        sm_s = cmp_.tile([128, NI], BF, tag=f"sm{h}",
                                         name=f"sm{h}")
                        nc.sync.dma_start(sm_s[:], A["sm"][h])
                        cm_ts.append((cm_s, sm_s))
                    morder = (0, 2, 1, 3, 4, 5)

                done = set()
                for m in morder:
                    pss = [psc.tile([128, 512], F32, tag="scps",
                                    name=f"qkvps_{s}_{m}_{i}")
                           for i in range(nch)]
                    for kc in range(KC):
                        for ci in range(nch):
                            cs = slice(ci * 512, (ci + 1) * 512)
                            nc.tensor.matmul(pss[ci][:],
                                             wbs[kc][:, m * 128:(m + 1) * 128],
                                             xts[kc][:, cs],
                                             start=(kc == 0), stop=(kc == KC - 1))
                    for ci in range(nch):
                        cs = slice(ci * 512, (ci + 1) * 512)
                        h = m % 2
                        kind = m // 2  # 0=q 1=k 2=v
                        if kind == 0:
                            dtile = q_txt[h] if s == 0 else q_imgr[h]
                            dst = dtile[:, cs]
                        elif kind == 1:
                            dst = kcat[h][:, cs] if s == 0 else k_imgr[h][:, cs]
                        else:
                            dst = vcat[h][:, cs] if s == 0 \
                                else vcat[h][:, L + ci * 512:L + (ci + 1) * 512]
                        nc.scalar.add(dst, pss[ci][:], qb_s[s][:, m:m + 1])
                    done.add(m)
                    if s == 1:
                        for h in range(HPC):
                            if h in done and (2 + h) in done and ('r', h) not in done:
                                rope_head(h, *cm_ts[h])
                                done.add(('r', h))

        # ------------------- phase 3: attention ------------------------
        with ExitStack() as c3:
            vtp = c3.enter_context(tc.tile_pool(name="vtp", bufs=1))
            ptp = c3.enter_context(tc.tile_pool(name="ptp", bufs=2))
            prp = c3.enter_context(tc.tile_pool(name="prp", bufs=3))
            dnp = c3.enter_context(tc.tile_pool(name="dnp", bufs=3))

            wop = c3.enter_context(tc.tile_pool(name="wop", bufs=4))
            oev = c3.enter_context(tc.tile_pool(name="oev", bufs=4))
            wob_s = []
            for s in (0, 1):
                wo_d = A["wo_txt"] if s == 0 else A["wo_img"]
                wob = []
                for kc in range(HPC):
                    wb = wop.tile([128, HID], BF, tag="wob",
                                  name=f"wob_{s}_{kc}")
                    nc.sync.dma_start(wb[:], wo_d[kc * 128:(kc + 1) * 128, :])
                    wob.append(wb)
                wob_s.append(wob)

            vts = []
            for h in range(HPC):
                vt = vtp.tile([128, NKT, 128], BF, tag=f"vt{h}", name=f"vt{h}")
                for jb in range(NKT // 4):
                    tp_ps = ptt.tile([128, 4, 128], BF, tag="trps",
                                     name=f"vtps_{h}_{jb}")
                    for jj in range(4):
                        j = jb * 4 + jj
                        nc.tensor.transpose(tp_ps[:, jj, :],
                                            vcat[h][:, j * 128:(j + 1) * 128],
                                            ident_s[:])
                    nc.scalar.copy(vt[:, jb * 4:(jb + 1) * 4, :], tp_ps[:])
                vts.append(vt)

            for s in (0, 1):
                Tq = L if s == 0 else NI
                ocol0 = 0 if s == 0 else L
                for g in range(Tq // 512):
                    for h in range(HPC):
                        q_t = q_txt[h] if s == 0 else q_img[h]
                        pt = ptp.tile([128, NKT, 512], BF, tag="pt",
                                      name=f"pt_{s}_{g}_{h}")
                        denrow = dnp.tile([1, 512], F32, tag="denrow",
                                          name=f"denrow_{s}_{g}_{h}")
                        for qt in range(4):
                            qc = g * 512 + qt * 128
                            pr_t = prp.tile([128, T], BF, tag="pr",
                                            name=f"pr_{s}_{g}_{h}_{qt}")
                            den5 = dnp.tile([128, 5], F32, tag="den5",
                                            name=f"den5_{s}_{g}_{h}_{qt}")
                            if s == 0:
                                # txt scores are small (|s*S| < ~10): no
                                # max-subtraction needed for fp32 exp.
                                for b in range(5):
                                    scp = psc.tile([128, 512], F32, tag="scps",
                                                   name=f"scp_{s}_{g}_{h}_{qt}_{b}")
                                    nc.tensor.matmul(
                                        scp[:], q_t[:, qc:qc + 128],
                                        kcat[h][:, b * 512:(b + 1) * 512],
                                        start=True, stop=True)
                                    nc.scalar.activation(
                                        pr_t[:, b * 512:(b + 1) * 512], scp[:],
                                        AF.Exp, bias=0.0, scale=SCALE,
                                        accum_out=den5[:, b:b + 1])
                            else:
                                # per-block softmax: block max -> exp with
                                # block bias (score psum frees right away),
                                # then correction cf_b = exp(s2*(mx_b - mx)).
                                mx5 = dnp.tile([128, 5], F32, tag="mx5",
                                               name=f"mx5_{s}_{g}_{h}_{qt}")
                                for b in range(5):
                                    scp = psc.tile([128, 512], F32, tag="scps",
                                                   name=f"scp_{s}_{g}_{h}_{qt}_{b}")
                                    nc.tensor.matmul(
                                        scp[:], q_t[:, qc:qc + 128],
                                        kcat[h][:, b * 512:(b + 1) * 512],
                                        start=True, stop=True)
                                    nc.vector.tensor_reduce(
                                        mx5[:, b:b + 1], scp[:],
                                        axis=AX.X, op=ALU.max)
                                    nbb = dnp.tile([128, 1], F32, tag="nbb",
                                                   name=f"nbb_{s}_{g}_{h}_{qt}_{b}")
                                    nc.vector.tensor_tensor(
                                        nbb[:], mx5[:, b:b + 1], ns2_s[:],
                                        ALU.mult)
                                    nc.scalar.activation(
                                        pr_t[:, b * 512:(b + 1) * 512], scp[:],
                                        AF.Exp, bias=nbb[:], scale=s2_s[:],
                                        accum_out=den5[:, b:b + 1])
                                mx = dnp.tile([128, 1], F32, tag="mx",
                                              name=f"mx_{s}_{g}_{h}_{qt}")
                                nc.vector.tensor_reduce(mx[:], mx5[:, 0:5],
                                                        axis=AX.X, op=ALU.max)
                                dm = dnp.tile([128, 5], F32, tag="dm",
                                              name=f"dm_{s}_{g}_{h}_{qt}")
                                nc.vector.tensor_scalar(
                                    dm[:], mx5[:], mx[:], None,
                                    ALU.subtract)
                                cf = dnp.tile([128, 5], F32, tag="cf",
                                              name=f"cf_{s}_{g}_{h}_{qt}")
                                nc.scalar.activation(cf[:], dm[:], AF.Exp,
                                                     bias=0.0, scale=s2_s[:])
                                for b in range(5):
                                    pb = pr_t[:, b * 512:(b + 1) * 512]
                                    if b % 2 == 0:
                                        nc.vector.tensor_scalar_mul(
                                            pb, pb, cf[:, b:b + 1])
                                    else:
                                        nc.scalar.mul(pb, pb, cf[:, b:b + 1])
                                nc.vector.tensor_tensor(den5[:], den5[:],
                                                        cf[:], ALU.mult)
                            den = dnp.tile([128, 1], F32, tag="den",
                                           name=f"den_{s}_{g}_{h}_{qt}")
                            nc.vector.tensor_reduce(den[:], den5[:, 0:5],
                                                    axis=AX.X, op=ALU.add)
                            rden = dnp.tile([128, 1], F32, tag="rden",
                                            name=f"rden_{s}_{g}_{h}_{qt}")
                            nc.vector.reciprocal(rden[:], den[:])
                            rd_ps = prd.tile([1, 128], F32, tag="rdps",
                                             name=f"rdps_{s}_{g}_{h}_{qt}")
                            nc.tensor.transpose(rd_ps[:], rden[:], identf_s[:])
                            nc.vector.tensor_copy(
                                denrow[0:1, qt * 128:(qt + 1) * 128],
                                rd_ps[0:1, :])
                            for jb in range(NKT // 4):
                                tp_ps = ptt.tile([128, 4, 128], BF, tag="trps",
                                                 name=f"tp_{s}_{g}_{h}_{qt}_{jb}")
                                for jj in range(4):
                                    j = jb * 4 + jj
                                    nc.tensor.transpose(
                                        tp_ps[:, jj, :],
                                        pr_t[:, j * 128:(j + 1) * 128],
                                        ident_s[:])
                                dst = pt[:, jb * 4:(jb + 1) * 4,
                                         qt * 128:(qt + 1) * 128]
                                if jb % 2 == 0:
                                    nc.scalar.copy(dst, tp_ps[:])
                                else:
                                    nc.vector.tensor_copy(dst, tp_ps[:])
                        rdenb = dnp.tile([128, 512], F32, tag="rdenb",
                                         name=f"rdenb_{s}_{g}_{h}")
                        nc.gpsimd.partition_broadcast(rdenb[:], denrow[0:1, :])
                        pv_ps = pacc.tile([128, 512], F32, tag="acc",
                                          name=f"pv_{s}_{g}_{h}")
                        for j in range(NKT):
                            nc.tensor.matmul(pv_ps[:], vts[h][:, j, :],
                                             pt[:, j, :],
                                             start=(j == 0), stop=(j == NKT - 1))
                        oc = slice(ocol0 + g * 512, ocol0 + (g + 1) * 512)
                        nc.vector.tensor_tensor(att[h][:, oc], pv_ps[:],
                                                rdenb[:], ALU.mult)
                    # out-proj for this group's columns (both heads ready)
                    for m in range(KC):
                        ps_o = prd.tile([128, 512], F32, tag="rdps",
                                        name=f"ops_{s}_{g}_{m}")
                        for kc in range(HPC):
                            nc.tensor.matmul(ps_o[:],
                                             wob_s[s][kc][:, m * 128:(m + 1) * 128],
                                             att[kc][:, oc],
                                             start=(kc == 0),
                                             stop=(kc == HPC - 1))
                        ev = oev.tile([128, 512], F32, tag="oevt")
                        if m % 2 == 0:
                            nc.vector.tensor_copy(ev[:], ps_o[:])
                        else:
                            nc.scalar.copy(ev[:], ps_o[:])
                        nc.sync.dma_start(
                            A["part1"][m * 128:(m + 1) * 128, oc], ev[:])


# --------------------------------------------------------------------------
# L2: fc1 + gelu + fc2
# --------------------------------------------------------------------------

def build_l2():
    nc = bacc.Bacc("TRN2", target_bir_lowering=False, debug=False)
    A = {}
    A["x2t"] = _din(nc, "x2t", [HID, T])
    A["w1_txt"] = _din(nc, "w1_txt", [HID, MH // NC])
    A["w1_img"] = _din(nc, "w1_img", [HID, MH // NC])
    A["b1_txt"] = _din(nc, "b1_txt", [128, 8], F32)
    A["b1_img"] = _din(nc, "b1_img", [128, 8], F32)
    A["w2_txt"] = _din(nc, "w2_txt", [MH // NC, HID])
    A["w2_img"] = _din(nc, "w2_img", [MH // NC, HID])
    A["part2"] = _dout(nc, "part2", [HID, T])

    with tile.TileContext(nc) as tc:
        _l2_body(tc, A)
    nc.compile()
    return nc


def _l2_body(tc, A):
    nc = tc.nc
    MS = MH // NC            # 1024 mlp cols per core
    M1 = MS // 128           # 8 fc1 m-tiles
    with ExitStack() as ctx:
        # static psum: 4 + 4 = 8 banks
        pf1 = ctx.enter_context(tc.tile_pool(name="pf1", bufs=4, space="PSUM"))
        pf2 = ctx.enter_context(tc.tile_pool(name="pf2", bufs=4, space="PSUM"))

        cp = ctx.enter_context(tc.tile_pool(name="const2", bufs=1))
        hp = ctx.enter_context(tc.tile_pool(name="hp", bufs=M1))
        xp = ctx.enter_context(tc.tile_pool(name="xt2", bufs=KC))
        w1p = ctx.enter_context(tc.tile_pool(name="w1b", bufs=KC))
        w2p = ctx.enter_context(tc.tile_pool(name="w2b", bufs=M1))
        oev = ctx.enter_context(tc.tile_pool(name="oev2", bufs=4))

        b1_s = []
        for s in (0, 1):
            t = cp.tile([128, M1], F32, tag=f"b1{s}")
            nc.sync.dma_start(t[:], A["b1_txt" if s == 0 else "b1_img"][:, :])
            b1_s.append(t)

        for s in (0, 1):
            Tn = L if s == 0 else NI
            col0 = 0 if s == 0 else L
            nch = Tn // 512
            w1_d = A["w1_txt"] if s == 0 else A["w1_img"]
            w2_d = A["w2_txt"] if s == 0 else A["w2_img"]

            xts = []
            for kc in range(KC):
                xtile = xp.tile([128, NI], BF, tag="x2")
                nc.sync.dma_start(xtile[:, 0:Tn],
                                  A["x2t"][kc * 128:(kc + 1) * 128,
                                           col0:col0 + Tn])
                xts.append(xtile)
            w1bs = []
            for kc in range(KC):
                wb = w1p.tile([128, MS], BF, tag="w1")
                nc.sync.dma_start(wb[:], w1_d[kc * 128:(kc + 1) * 128, :])
                w1bs.append(wb)

            hts = []
            for m in range(M1):
                pss = [pf1.tile([128, 512], F32, tag="f1", name=f"f1_{s}_{m}_{i}") for i in range(nch)]
                for kc in range(KC):
                    for ci in range(nch):
                        cs = slice(ci * 512, (ci + 1) * 512)
                        nc.tensor.matmul(pss[ci][:],
                                         w1bs[kc][:, m * 128:(m + 1) * 128],
                                         xts[kc][:, cs],
                                         start=(kc == 0), stop=(kc == KC - 1))
                ht = hp.tile([128, NI], BF, tag="h")
                for ci in range(nch):
                    cs = slice(ci * 512, (ci + 1) * 512)
                    nc.scalar.activation(ht[:, cs], pss[ci][:],
                                         AF.Gelu_apprx_tanh,
                                         bias=b1_s[s][:, m:m + 1], scale=1.0)
                hts.append(ht)

            w2bs = []
            for kc in range(M1):
                wb = w2p.tile([128, HID], BF, tag="w2")
                nc.sync.dma_start(wb[:], w2_d[kc * 128:(kc + 1) * 128, :])
                w2bs.append(wb)

            for m2 in range(KC):
                pss = [pf2.tile([128, 512], F32, tag="f2", name=f"f2_{s}_{m2}_{i}") for i in range(nch)]
                for kc in range(M1):
                    for ci in range(nch):
                        cs = slice(ci * 512, (ci + 1) * 512)
                        nc.tensor.matmul(pss[ci][:],
                                         w2bs[kc][:, m2 * 128:(m2 + 1) * 128],
                                         hts[kc][:, cs],
                                         start=(kc == 0), stop=(kc == M1 - 1))
                for ci in range(nch):
                    gcs = slice(col0 + ci * 512, col0 + (ci + 1) * 512)
                    ev = oev.tile([128, 512], F32, tag="oevt2")
                    if ci % 2 == 0:
                        nc.vector.tensor_copy(ev[:], pss[ci][:])
                    else:
                        nc.scalar.copy(ev[:], pss[ci][:])
                    nc.sync.dma_start(A["part2"][m2 * 128:(m2 + 1) * 128, gcs],
                                      ev[:])


# --------------------------------------------------------------------------
# host math
# --------------------------------------------------------------------------

def _silu(x):
    return x / (1.0 + np.exp(-x))


def _resize_bilinear_8x8(a, H, W):
    """jax.image.resize(..., 'bilinear') equivalent (half-pixel centers)."""
    ih, iw = a.shape
    y = (np.arange(H, dtype=np.float64) + 0.5) * (ih / H) - 0.5
    x = (np.arange(W, dtype=np.float64) + 0.5) * (iw / W) - 0.5
    y0 = np.floor(y).astype(np.int64)
    x0 = np.floor(x).astype(np.int64)
    wy = (y - y0).astype(np.float32)
    wx = (x - x0).astype(np.float32)
    y0c = np.clip(y0, 0, ih - 1)
    y1c = np.clip(y0 + 1, 0, ih - 1)
    x0c = np.clip(x0, 0, iw - 1)
    x1c = np.clip(x0 + 1, 0, iw - 1)
    a = a.astype(np.float32)
    top = a[y0c][:, x0c] * (1 - wx)[None, :] + a[y0c][:, x1c] * wx[None, :]
    bot = a[y1c][:, x0c] * (1 - wx)[None, :] + a[y1c][:, x1c] * wx[None, :]
    return top * (1 - wy)[:, None] + bot * wy[:, None]


_PROGS = {}


def _get_progs():
    if "l1" not in _PROGS:
        _PROGS["l1"] = build_l1()
        _PROGS["l2"] = build_l2()
    return _PROGS["l1"], _PROGS["l2"]


def _run(nc, in_maps, trace=False):
    return bass_utils.run_bass_kernel_spmd(
        nc, in_maps, core_ids=list(range(NC)), trace=trace)


def kernel(txt, img, vec, rope, sol_temperature, sol_spatial,
           ada_img_w, ada_img_b, ada_img_nw, ada_txt_w, ada_txt_b, ada_txt_nw,
           txt_qkv_w, img_qkv_w, txt_out_w, img_out_w, mod_w, mod_b,
           img_n2_w, txt_n2_w, img_fc1_w, img_fc1_b, img_fc2_w, img_fc2_b,
           txt_fc1_w, txt_fc1_b, txt_fc2_w, txt_fc2_b, spatial_h, spatial_w,
           _trace=False, _timing=None):
    f = np.float32
    nc1, nc2 = _get_progs()

    x_txt = np.asarray(txt, f)[0]            # [L, HID]
    x_img = np.asarray(img, f)[0]            # [NI, HID]
    v = np.asarray(vec, f)[0]                # [HID]

    sv = _silu(v)
    e_txt = sv @ np.asarray(ada_txt_w, f) + np.asarray(ada_txt_b, f)
    e_img = sv @ np.asarray(ada_img_w, f) + np.asarray(ada_img_b, f)
    shm_t, scm_t, gm_t, shp_t, scp_t, gp_t = np.split(e_txt, 6)
    shm_i, scm_i, gm_i, shp_i, scp_i, gp_i = np.split(e_img, 6)

    a1_t = np.asarray(ada_txt_nw, f) * (1.0 + scm_t)
    a1_i = np.asarray(ada_img_nw, f) * (1.0 + scm_i)

    rs1_t = 1.0 / np.sqrt((x_txt * x_txt).mean(-1) + EPS)     # [L]
    rs1_i = 1.0 / np.sqrt((x_img * x_img).mean(-1) + EPS)     # [NI]

    # spatial modulation folded into rope cos/sin
    up = _resize_bilinear_8x8(np.asarray(sol_spatial, f)[0], 64, 32).reshape(-1)
    mod = np.exp(np.clip(up[None, :] * np.asarray(mod_w, f)[:, None]
                         + np.asarray(mod_b, f)[:, None], -2.0, 2.0))  # [NH, NI]
    fr = np.asarray(rope, f)[:, 0]                        # [NI, HD]
    cos = np.repeat(fr[:, 0::2], 2, axis=1)               # [NI, HD]
    sin = np.repeat(fr[:, 1::2], 2, axis=1)

    temp = max(float(np.asarray(sol_temperature, f).mean()), 0.1)
    s2v = SCALE / temp
    s2 = np.full((128, 1), s2v, f)
    ns2 = -s2

    permT = np.zeros((128, 128), f)
    for i in range(64):
        permT[2 * i + 1, 2 * i] = -1.0
        permT[2 * i, 2 * i + 1] = 1.0
    ident = np.eye(128, dtype=f)

    wq_t_full = np.asarray(txt_qkv_w, f)
    wq_i_full = np.asarray(img_qkv_w, f)
    wo_t_full = np.asarray(txt_out_w, f)
    wo_i_full = np.asarray(img_out_w, f)

    xt_txt = np.ascontiguousarray(x_txt.T * rs1_t[None, :]).astype(BF16)
    xt_img = np.ascontiguousarray(x_img.T * rs1_i[None, :]).astype(BF16)

    in_maps1 = []
    for c in range(NC):
        h0, h1 = 2 * c, 2 * c + 1
        cols = np.concatenate([
            np.arange(h0 * HD, (h0 + 1) * HD),
            np.arange(h1 * HD, (h1 + 1) * HD),
            HID + np.arange(h0 * HD, (h0 + 1) * HD),
            HID + np.arange(h1 * HD, (h1 + 1) * HD),
            2 * HID + np.arange(h0 * HD, (h0 + 1) * HD),
            2 * HID + np.arange(h1 * HD, (h1 + 1) * HD)])
        wq_t_raw = wq_t_full[:, cols]
        wq_i_raw = wq_i_full[:, cols]
        cm = np.stack([(cos * mod[h][:, None]).T for h in (h0, h1)]).astype(BF16)
        sm = np.stack([(sin * mod[h][:, None]).T for h in (h0, h1)]).astype(BF16)
        in_maps1.append({
            "xt_txt": xt_txt, "xt_img": xt_img,
            "wq_txt": (a1_t[:, None] * wq_t_raw).astype(BF16),
            "wq_img": (a1_i[:, None] * wq_i_raw).astype(BF16),
            "qb_txt": np.ascontiguousarray(
                (shm_t @ wq_t_raw).astype(f).reshape(6, 128).T),
            "qb_img": np.ascontiguousarray(
                (shm_i @ wq_i_raw).astype(f).reshape(6, 128).T),
            "cm": cm, "sm": sm,
            "permT": permT.astype(BF16), "ident": ident.astype(BF16),
            "identf": ident, "s2": s2, "ns2": ns2,
            "wo_txt": wo_t_full[c * 256:(c + 1) * 256, :].astype(BF16),
            "wo_img": wo_i_full[c * 256:(c + 1) * 256, :].astype(BF16),
        })

    res1 = _run(nc1, in_maps1, trace=_trace)
    if _timing is not None:
        _timing.append(("l1", res1.exec_time_ns))

    attnT = res1.results[0]["part1"].astype(f)
    for c in range(1, NC):
        attnT += res1.results[c]["part1"]

    x2_t = x_txt.T + gm_t[:, None] * attnT[:, :L]     # [HID, L]
    x2_i = x_img.T + gm_i[:, None] * attnT[:, L:]     # [HID, NI]

    rs2_t = 1.0 / np.sqrt((x2_t * x2_t).mean(0) + EPS)
    rs2_i = 1.0 / np.sqrt((x2_i * x2_i).mean(0) + EPS)
    x2t = np.concatenate([x2_t * rs2_t[None, :],
                          x2_i * rs2_i[None, :]], axis=1).astype(BF16)

    a2_t = np.asarray(txt_n2_w, f) * (1.0 + scp_t)
    a2_i = np.asarray(img_n2_w, f) * (1.0 + scp_i)
    b1f_t = shp_t @ np.asarray(txt_fc1_w, f) + np.asarray(txt_fc1_b, f)
    b1f_i = shp_i @ np.asarray(img_fc1_w, f) + np.asarray(img_fc1_b, f)

    MS = MH // NC
    in_maps2 = []
    for c in range(NC):
        csl = slice(c * MS, (c + 1) * MS)
        in_maps2.append({
            "x2t": x2t,
            "w1_txt": (a2_t[:, None]
                       * np.asarray(txt_fc1_w, f)[:, csl]).astype(BF16),
            "w1_img": (a2_i[:, None]
                       * np.asarray(img_fc1_w, f)[:, csl]).astype(BF16),
            "b1_txt": np.ascontiguousarray(b1f_t[csl].reshape(8, 128).T.astype(f)),
            "b1_img": np.ascontiguousarray(b1f_i[csl].reshape(8, 128).T.astype(f)),
            "w2_txt": np.asarray(txt_fc2_w, f)[csl, :].astype(BF16),
            "w2_img": np.asarray(img_fc2_w, f)[csl, :].astype(BF16),
        })

    res2 = _run(nc2, in_maps2, trace=_trace)
    if _timing is not None:
        _timing.append(("l2", res2.exec_time_ns))

    mlpT = res2.results[0]["part2"].astype(f)
    for c in range(1, NC):
        mlpT += res2.results[c]["part2"]

    out_t = x2_t + gp_t[:, None] * (mlpT[:, :L]
                                    + np.asarray(txt_fc2_b, f)[:, None])
    out_i = x2_i + gp_i[:, None] * (mlpT[:, L:]
                                    + np.asarray(img_fc2_b, f)[:, None])

    txt_out = np.ascontiguousarray(out_t.T).reshape(1, L, HID).astype(f)
    img_out = np.ascontiguousarray(out_i.T).reshape(1, NI, HID).astype(f)
    return txt_out, img_out


# revision 20
# speedup vs baseline: 1.2967x; 1.0248x over previous
"""DoubleStreamBlock (MMDiT-style) Trainium2 kernel, 8 NeuronCores.

Sharding: tensor-parallel over the 16 attention heads (2 heads/core);
qkv/fc1 column-sharded, out/fc2 row-sharded.  Two device launches with
host-side partial-sum reduction between them (on-device AllReduce of the
21MB activations would be slower than the compute itself):

  L1: qkv projection (+folded adaLN norm), RoPE+spatial modulation,
      attention (2 heads/core over the full txt+img sequence),
      row-sharded output projection -> per-core partial [HID, T].
  host: sum partials, attention residual, norm2 stats.
  L2: fc1 (+folded norm2) -> gelu -> fc2 row-sharded -> partials.
  host: sum partials, final residual.

Activations live transposed on device: [feature, token] so every matmul
is lhsT=W-tile (stationary), rhs=activation (moving), and attention
q/k/v arrive as [head_dim, token] with no transposes.

PSUM discipline: all pools are allocated once at kernel top and fit in
the 8 banks statically (phase-scoped PSUM pools deadlock the tile
scheduler via released-zone overlap deps).
"""

import sys
import os

for _p in ("/opt/trn_rl_repo",):
    if _p not in sys.path:
        sys.path.insert(0, _p)

import numpy as np
import ml_dtypes
from contextlib import ExitStack

import concourse.bass as bass
import concourse.tile as tile
from concourse import bacc, mybir
from concourse import bass_utils

BF16 = ml_dtypes.bfloat16
BF = mybir.dt.bfloat16
F32 = mybir.dt.float32
AF = mybir.ActivationFunctionType
ALU = mybir.AluOpType
AX = mybir.AxisListType

L, NI, T = 512, 2048, 2560          # txt tokens, img tokens, total
HID, NH, HD, MH = 2048, 16, 128, 8192
NC, HPC = 8, 2                       # cores, heads per core
SCALE = float(HD) ** -0.5
EPS = 1e-6
KC = HID // 128                      # 16 contraction chunks
NKT = T // 128                       # 20 key tiles


def _din(nc, name, shape, dt=BF):
    return nc.dram_tensor(name, shape, dt, kind="ExternalInput").ap()


def _dout(nc, name, shape, dt=F32):
    return nc.dram_tensor(name, shape, dt, kind="ExternalOutput").ap()


# --------------------------------------------------------------------------
# L1: qkv + rope/mod + attention + out-proj
# --------------------------------------------------------------------------

def build_l1():
    nc = bacc.Bacc("TRN2", target_bir_lowering=False, debug=False)
    A = {}
    A["xt_txt"] = _din(nc, "xt_txt", [HID, L])
    A["xt_img"] = _din(nc, "xt_img", [HID, NI])
    A["wq_txt"] = _din(nc, "wq_txt", [HID, 768])
    A["wq_img"] = _din(nc, "wq_img", [HID, 768])
    A["qb_txt"] = _din(nc, "qb_txt", [128, 6], F32)
    A["qb_img"] = _din(nc, "qb_img", [128, 6], F32)
    A["cm"] = _din(nc, "cm", [HPC, 128, NI])
    A["sm"] = _din(nc, "sm", [HPC, 128, NI])
    A["permT"] = _din(nc, "permT", [128, 128])
    A["ident"] = _din(nc, "ident", [128, 128])
    A["identf"] = _din(nc, "identf", [128, 128], F32)
    A["s2"] = _din(nc, "s2", [128, 1], F32)
    A["ns2"] = _din(nc, "ns2", [128, 1], F32)
    A["wo_txt"] = _din(nc, "wo_txt", [HPC * HD, HID])
    A["wo_img"] = _din(nc, "wo_img", [HPC * HD, HID])
    A["part1"] = _dout(nc, "part1", [HID, T])

    with tile.TileContext(nc) as tc:
        _l1_body(tc, A)
    nc.compile()
    return nc


def _l1_body(tc, A):
    nc = tc.nc
    with ExitStack() as ctx:
        # ---- static psum pools: 1 + 4 + 2 + 1 = 8 banks.  Scores use
        # per-block softmax (block max + correction factors) so score psum
        # frees right after its block exp; transposes get their own pool.
        pacc = ctx.enter_context(tc.tile_pool(name="pacc", bufs=1, space="PSUM"))
        psc = ctx.enter_context(tc.tile_pool(name="psc", bufs=4, space="PSUM"))
        ptt = ctx.enter_context(tc.tile_pool(name="ptt", bufs=2, space="PSUM"))
        prd = ctx.enter_context(tc.tile_pool(name="prd", bufs=1, space="PSUM"))

        const = ctx.enter_context(tc.tile_pool(name="const", bufs=1))
        qkvp = ctx.enter_context(tc.tile_pool(name="qkvp", bufs=1))
        attp = ctx.enter_context(tc.tile_pool(name="attp", bufs=1))

        ident_s = const.tile([128, 128], BF, tag="ident")
        identf_s = const.tile([128, 128], F32, tag="identf")
        permT_s = const.tile([128, 128], BF, tag="permT")
        s2_s = const.tile([128, 1], F32, tag="s2")
        ns2_s = const.tile([128, 1], F32, tag="ns2")
        qb_s = []
        for s in (0, 1):
            t = const.tile([128, 6], F32, tag=f"qb{s}")
            nc.sync.dma_start(t[:], A["qb_txt" if s == 0 else "qb_img"][:, :])
            qb_s.append(t)

        # persistent q/k/v and attention-out tiles (bf16, [128, tokens])
        q_txt = [qkvp.tile([128, L], BF, tag=f"qtxt{h}", name=f"qtxt{h}")
                 for h in range(HPC)]
        q_imgr = [qkvp.tile([128, NI], BF, tag=f"qimgr{h}", name=f"qimgr{h}")
                  for h in range(HPC)]
        q_img = [qkvp.tile([128, NI], BF, tag=f"qimg{h}", name=f"qimg{h}")
                 for h in range(HPC)]
        k_imgr = [qkvp.tile([128, NI], BF, tag=f"kimgr{h}", name=f"kimgr{h}")
                  for h in range(HPC)]
        kcat = [qkvp.tile([128, T], BF, tag=f"kcat{h}", name=f"kcat{h}")
                for h in range(HPC)]
        vcat = [qkvp.tile([128, T], BF, tag=f"vcat{h}", name=f"vcat{h}")
                for h in range(HPC)]
        att = [attp.tile([128, T], BF, tag=f"att{h}", name=f"att{h}")
               for h in range(HPC)]

        # ------------------- phase 1: qkv projection -------------------
        # img stream m-order computes q then k per head so RoPE overlaps the
        # remaining qkv matmuls; v comes last.
        with ExitStack() as c1:
            xp = c1.enter_context(tc.tile_pool(name="xt1", bufs=16))
            wp = c1.enter_context(tc.tile_pool(name="wb1", bufs=16))
            cmp_ = c1.enter_context(tc.tile_pool(name="cmsm", bufs=1))
            rtt = c1.enter_context(tc.tile_pool(name="tmp_rope", bufs=4))

            def rope_head(h, cm_s, sm_s):
                for src_t, dsttile, d0 in ((q_imgr[h], q_img[h], 0),
                                           (k_imgr[h], kcat[h], L)):
                    for ci in range(NI // 512):
                        cs = slice(ci * 512, (ci + 1) * 512)
                        ds = slice(d0 + ci * 512, d0 + (ci + 1) * 512)
                        rps = pacc.tile([128, 512], F32, tag="acc")
                        nc.tensor.matmul(rps[:], permT_s[:], src_t[:, cs],
                                         start=True, stop=True)
                        t1 = rtt.tile([128, 512], BF, tag="t1")
                        nc.vector.tensor_tensor(t1[:], src_t[:, cs],
                                                cm_s[:, cs], ALU.mult)
                        t2 = rtt.tile([128, 512], BF, tag="t2")
                        nc.vector.tensor_tensor(t2[:], rps[:], sm_s[:, cs],
                                                ALU.mult)
                        nc.vector.tensor_tensor(dsttile[:, ds], t1[:], t2[:],
                                                ALU.add)

            for s in (0, 1):
                Tn = L if s == 0 else NI
                col0 = 0 if s == 0 else L
                nch = Tn // 512
                xt_d = A["xt_txt"] if s == 0 else A["xt_img"]
                wq_d = A["wq_txt"] if s == 0 else A["wq_img"]

                wbs, xts = [], []
                for kc in range(KC):
                    wb = wp.tile([128, 768], BF, tag="wb")
                    nc.sync.dma_start(wb[:], wq_d[kc * 128:(kc + 1) * 128, :])
                    wbs.append(wb)
                    xtile = xp.tile([128, NI], BF, tag="xch")
                    nc.sync.dma_start(xtile[:, 0:Tn],
                                      xt_d[kc * 128:(kc + 1) * 128, :])
                    xts.append(xtile)
                if s == 0:
                    # consts ride behind the first stream's operands
                    nc.sync.dma_start(ident_s[:], A["ident"][:, :])
                    nc.sync.dma_start(identf_s[:], A["identf"][:, :])
                    nc.sync.dma_start(permT_s[:], A["permT"][:, :])
                    nc.sync.dma_start(s2_s[:], A["s2"][:, :])
                    nc.sync.dma_start(ns2_s[:], A["ns2"][:, :])
                    morder = range(6)
                    cm_ts = None
                else:
                    cm_ts = []
                    for h in range(HPC):
                        cm_s = cmp_.tile([128, NI], BF, tag=f"cm{h}",
                                         name=f"cm{h}")
                        nc.sync.dma_start(cm_s[:], A["cm"][h])
                        sm_s = cmp_.tile([128, NI], BF, tag=f"sm{h}",
                                         name=f"sm{h}")
                        nc.sync.dma_start(sm_s[:], A["sm"][h])
                        cm_ts.append((cm_s, sm_s))
                    morder = (0, 2, 1, 3, 4, 5)

                done = set()
                for m in morder:
                    pss = [psc.tile([128, 512], F32, tag="scps",
                                    name=f"qkvps_{s}_{m}_{i}")
                           for i in range(nch)]
                    for kc in range(KC):
                        for ci in range(nch):
                            cs = slice(ci * 512, (ci + 1) * 512)
                            nc.tensor.matmul(pss[ci][:],
                                             wbs[kc][:, m * 128:(m + 1) * 128],
                                             xts[kc][:, cs],
                                             start=(kc == 0), stop=(kc == KC - 1))
                    for ci in range(nch):
                        cs = slice(ci * 512, (ci + 1) * 512)
                        h = m % 2
                        kind = m // 2  # 0=q 1=k 2=v
                        if kind == 0:
                            dtile = q_txt[h] if s == 0 else q_imgr[h]
                            dst = dtile[:, cs]
                        elif kind == 1:
                            dst = kcat[h][:, cs] if s == 0 else k_imgr[h][:, cs]
                        else:
                            dst = vcat[h][:, cs] if s == 0 \
                                else vcat[h][:, L + ci * 512:L + (ci + 1) * 512]
                        if (m + ci) % 2 == 0:
                            nc.scalar.add(dst, pss[ci][:],
                                          qb_s[s][:, m:m + 1])
                        else:
                            nc.vector.tensor_scalar(
                                dst, pss[ci][:], qb_s[s][:, m:m + 1], None,
                                ALU.add)
                    done.add(m)
                    if s == 1:
                        for h in range(HPC):
                            if h in done and (2 + h) in done and ('r', h) not in done:
                                rope_head(h, *cm_ts[h])
                                done.add(('r', h))

        # ------------------- phase 3: attention ------------------------
        with ExitStack() as c3:
            vtp = c3.enter_context(tc.tile_pool(name="vtp", bufs=1))
            ptp = c3.enter_context(tc.tile_pool(name="ptp", bufs=2))
            prp = c3.enter_context(tc.tile_pool(name="prp", bufs=3))
            dnp = c3.enter_context(tc.tile_pool(name="dnp", bufs=3))

            wop = c3.enter_context(tc.tile_pool(name="wop", bufs=4))
            oev = c3.enter_context(tc.tile_pool(name="oev", bufs=4))
            wob_s = []
            for s in (0, 1):
                wo_d = A["wo_txt"] if s == 0 else A["wo_img"]
                wob = []
                for kc in range(HPC):
                    wb = wop.tile([128, HID], BF, tag="wob",
                                  name=f"wob_{s}_{kc}")
                    nc.sync.dma_start(wb[:], wo_d[kc * 128:(kc + 1) * 128, :])
                    wob.append(wb)
                wob_s.append(wob)

            vts = []
            for h in range(HPC):
                vt = vtp.tile([128, NKT, 128], BF, tag=f"vt{h}", name=f"vt{h}")
                for jb in range(NKT // 4):
                    tp_ps = ptt.tile([128, 4, 128], BF, tag="trps",
                                     name=f"vtps_{h}_{jb}")
                    for jj in range(4):
                        j = jb * 4 + jj
                        nc.tensor.transpose(tp_ps[:, jj, :],
                                            vcat[h][:, j * 128:(j + 1) * 128],
                                            ident_s[:])
                    nc.scalar.copy(vt[:, jb * 4:(jb + 1) * 4, :], tp_ps[:])
                vts.append(vt)

            for s in (0, 1):
                Tq = L if s == 0 else NI
                ocol0 = 0 if s == 0 else L
                for g in range(Tq // 512):
                    for h in range(HPC):
                        q_t = q_txt[h] if s == 0 else q_img[h]
                        pt = ptp.tile([128, NKT, 512], BF, tag="pt",
                                      name=f"pt_{s}_{g}_{h}")
                        denrow = dnp.tile([1, 512], F32, tag="denrow",
                                          name=f"denrow_{s}_{g}_{h}")
                        for qt in range(4):
                            qc = g * 512 + qt * 128
                            pr_t = prp.tile([128, T], BF, tag="pr",
                                            name=f"pr_{s}_{g}_{h}_{qt}")
                            den5 = dnp.tile([128, 5], F32, tag="den5",
                                            name=f"den5_{s}_{g}_{h}_{qt}")
                            if s == 0:
                                # txt scores are small (|s*S| < ~10): no
                                # max-subtraction needed for fp32 exp.
                                for b in range(5):
                                    scp = psc.tile([128, 512], F32, tag="scps",
                                                   name=f"scp_{s}_{g}_{h}_{qt}_{b}")
                                    nc.tensor.matmul(
                                        scp[:], q_t[:, qc:qc + 128],
                                        kcat[h][:, b * 512:(b + 1) * 512],
                                        start=True, stop=True)
                                    nc.scalar.activation(
                                        pr_t[:, b * 512:(b + 1) * 512], scp[:],
                                        AF.Exp, bias=0.0, scale=SCALE,
                                        accum_out=den5[:, b:b + 1])
                            else:
                                # per-block softmax: block max -> exp with
                                # block bias (score psum frees right away),
                                # then correction cf_b = exp(s2*(mx_b - mx)).
                                mx5 = dnp.tile([128, 5], F32, tag="mx5",
                                               name=f"mx5_{s}_{g}_{h}_{qt}")
                                for b in range(5):
                                    scp = psc.tile([128, 512], F32, tag="scps",
                                                   name=f"scp_{s}_{g}_{h}_{qt}_{b}")
                                    nc.tensor.matmul(
                                        scp[:], q_t[:, qc:qc + 128],
                                        kcat[h][:, b * 512:(b + 1) * 512],
                                        start=True, stop=True)
                                    nc.vector.tensor_reduce(
                                        mx5[:, b:b + 1], scp[:],
                                        axis=AX.X, op=ALU.max)
                                    nbb = dnp.tile([128, 1], F32, tag="nbb",
                                                   name=f"nbb_{s}_{g}_{h}_{qt}_{b}")
                                    nc.vector.tensor_tensor(
                                        nbb[:], mx5[:, b:b + 1], ns2_s[:],
                                        ALU.mult)
                                    nc.scalar.activation(
                                        pr_t[:, b * 512:(b + 1) * 512], scp[:],
                                        AF.Exp, bias=nbb[:], scale=s2_s[:],
                                        accum_out=den5[:, b:b + 1])
                                mx = dnp.tile([128, 1], F32, tag="mx",
                                              name=f"mx_{s}_{g}_{h}_{qt}")
                                nc.vector.tensor_reduce(mx[:], mx5[:, 0:5],
                                                        axis=AX.X, op=ALU.max)
                                dm = dnp.tile([128, 5], F32, tag="dm",
                                              name=f"dm_{s}_{g}_{h}_{qt}")
                                nc.vector.tensor_scalar(
                                    dm[:], mx5[:], mx[:], None,
                                    ALU.subtract)
                                cf = dnp.tile([128, 5], F32, tag="cf",
                                              name=f"cf_{s}_{g}_{h}_{qt}")
                                nc.scalar.activation(cf[:], dm[:], AF.Exp,
                                                     bias=0.0, scale=s2_s[:])
                                for b in range(5):
                                    pb = pr_t[:, b * 512:(b + 1) * 512]
                                    if b % 2 == 0:
                                        nc.vector.tensor_scalar_mul(
                                            pb, pb, cf[:, b:b + 1])
                                    else:
                                        nc.scalar.mul(pb, pb, cf[:, b:b + 1])
                                nc.vector.tensor_tensor(den5[:], den5[:],
                                                        cf[:], ALU.mult)
                            den = dnp.tile([128, 1], F32, tag="den",
                                           name=f"den_{s}_{g}_{h}_{qt}")
                            nc.vector.tensor_reduce(den[:], den5[:, 0:5],
                                                    axis=AX.X, op=ALU.add)
                            rden = dnp.tile([128, 1], F32, tag="rden",
                                            name=f"rden_{s}_{g}_{h}_{qt}")
                            nc.vector.reciprocal(rden[:], den[:])
                            rd_ps = prd.tile([1, 128], F32, tag="rdps",
                                             name=f"rdps_{s}_{g}_{h}_{qt}")
                            nc.tensor.transpose(rd_ps[:], rden[:], identf_s[:])
                            nc.vector.tensor_copy(
                                denrow[0:1, qt * 128:(qt + 1) * 128],
                                rd_ps[0:1, :])
                            for jb in range(NKT // 4):
                                tp_ps = ptt.tile([128, 4, 128], BF, tag="trps",
                                                 name=f"tp_{s}_{g}_{h}_{qt}_{jb}")
                                for jj in range(4):
                                    j = jb * 4 + jj
                                    nc.tensor.transpose(
                                        tp_ps[:, jj, :],
                                        pr_t[:, j * 128:(j + 1) * 128],
                                        ident_s[:])
                                dst = pt[:, jb * 4:(jb + 1) * 4,
                                         qt * 128:(qt + 1) * 128]
                                if jb % 2 == 0:
                                    nc.scalar.copy(dst, tp_ps[:])
                                else:
                                    nc.vector.tensor_copy(dst, tp_ps[:])
                        rdenb = dnp.tile([128, 512], F32, tag="rdenb",
                                         name=f"rdenb_{s}_{g}_{h}")
                        nc.gpsimd.partition_broadcast(rdenb[:], denrow[0:1, :])
                        pv_ps = pacc.tile([128, 512], F32, tag="acc",
                                          name=f"pv_{s}_{g}_{h}")
                        for j in range(NKT):
                            nc.tensor.matmul(pv_ps[:], vts[h][:, j, :],
                                             pt[:, j, :],
                                             start=(j == 0), stop=(j == NKT - 1))
                        oc = slice(ocol0 + g * 512, ocol0 + (g + 1) * 512)
                        nc.vector.tensor_tensor(att[h][:, oc], pv_ps[:],
                                                rdenb[:], ALU.mult)
                    # out-proj for this group's columns (both heads ready)
                    for m in range(KC):
                        ps_o = prd.tile([128, 512], F32, tag="rdps",
                                        name=f"ops_{s}_{g}_{m}")
                        for kc in range(HPC):
                            nc.tensor.matmul(ps_o[:],
                                             wob_s[s][kc][:, m * 128:(m + 1) * 128],
                                             att[kc][:, oc],
                                             start=(kc == 0),
                                             stop=(kc == HPC - 1))
                        ev = oev.tile([128, 512], F32, tag="oevt")
                        if m % 2 == 0:
                            nc.vector.tensor_copy(ev[:], ps_o[:])
                        else:
                            nc.scalar.copy(ev[:], ps_o[:])
                        nc.sync.dma_start(
                            A["part1"][m * 128:(m + 1) * 128, oc], ev[:])


# --------------------------------------------------------------------------
# L2: fc1 + gelu + fc2
# --------------------------------------------------------------------------

def build_l2():
    nc = bacc.Bacc("TRN2", target_bir_lowering=False, debug=False)
    A = {}
    A["x2t"] = _din(nc, "x2t", [HID, T])
    A["w1_txt"] = _din(nc, "w1_txt", [HID, MH // NC])
    A["w1_img"] = _din(nc, "w1_img", [HID, MH // NC])
    A["b1_txt"] = _din(nc, "b1_txt", [128, 8], F32)
    A["b1_img"] = _din(nc, "b1_img", [128, 8], F32)
    A["w2_txt"] = _din(nc, "w2_txt", [MH // NC, HID])
    A["w2_img"] = _din(nc, "w2_img", [MH // NC, HID])
    A["part2"] = _dout(nc, "part2", [HID, T])

    with tile.TileContext(nc) as tc:
        _l2_body(tc, A)
    nc.compile()
    return nc


def _l2_body(tc, A):
    nc = tc.nc
    MS = MH // NC            # 1024 mlp cols per core
    M1 = MS // 128           # 8 fc1 m-tiles
    with ExitStack() as ctx:
        # static psum: 4 + 4 = 8 banks
        pf1 = ctx.enter_context(tc.tile_pool(name="pf1", bufs=4, space="PSUM"))
        pf2 = ctx.enter_context(tc.tile_pool(name="pf2", bufs=4, space="PSUM"))

        cp = ctx.enter_context(tc.tile_pool(name="const2", bufs=1))
        hp = ctx.enter_context(tc.tile_pool(name="hp", bufs=M1))
        xp = ctx.enter_context(tc.tile_pool(name="xt2", bufs=KC))
        w1p = ctx.enter_context(tc.tile_pool(name="w1b", bufs=KC))
        w2p = ctx.enter_context(tc.tile_pool(name="w2b", bufs=M1))
        oev = ctx.enter_context(tc.tile_pool(name="oev2", bufs=4))

        b1_s = []
        for s in (0, 1):
            t = cp.tile([128, M1], F32, tag=f"b1{s}")
            nc.sync.dma_start(t[:], A["b1_txt" if s == 0 else "b1_img"][:, :])
            b1_s.append(t)

        for s in (0, 1):
            Tn = L if s == 0 else NI
            col0 = 0 if s == 0 else L
            nch = Tn // 512
            w1_d = A["w1_txt"] if s == 0 else A["w1_img"]
            w2_d = A["w2_txt"] if s == 0 else A["w2_img"]

            xts, w1bs = [], []
            for kc in range(KC):
                wb = w1p.tile([128, MS], BF, tag="w1")
                nc.sync.dma_start(wb[:], w1_d[kc * 128:(kc + 1) * 128, :])
                w1bs.append(wb)
                xtile = xp.tile([128, NI], BF, tag="x2")
                nc.sync.dma_start(xtile[:, 0:Tn],
                                  A["x2t"][kc * 128:(kc + 1) * 128,
                                           col0:col0 + Tn])
                xts.append(xtile)

            hts = []
            for m in range(M1):
                pss = [pf1.tile([128, 512], F32, tag="f1", name=f"f1_{s}_{m}_{i}") for i in range(nch)]
                for kc in range(KC):
                    for ci in range(nch):
                        cs = slice(ci * 512, (ci + 1) * 512)
                        nc.tensor.matmul(pss[ci][:],
                                         w1bs[kc][:, m * 128:(m + 1) * 128],
                                         xts[kc][:, cs],
                                         start=(kc == 0), stop=(kc == KC - 1))
                ht = hp.tile([128, NI], BF, tag="h")
                for ci in range(nch):
                    cs = slice(ci * 512, (ci + 1) * 512)
                    nc.scalar.activation(ht[:, cs], pss[ci][:],
                                         AF.Gelu_apprx_tanh,
                                         bias=b1_s[s][:, m:m + 1], scale=1.0)
                hts.append(ht)

            w2bs = []
            for kc in range(M1):
                wb = w2p.tile([128, HID], BF, tag="w2")
                nc.sync.dma_start(wb[:], w2_d[kc * 128:(kc + 1) * 128, :])
                w2bs.append(wb)

            for m2 in range(KC):
                pss = [pf2.tile([128, 512], F32, tag="f2", name=f"f2_{s}_{m2}_{i}") for i in range(nch)]
                for kc in range(M1):
                    for ci in range(nch):
                        cs = slice(ci * 512, (ci + 1) * 512)
                        nc.tensor.matmul(pss[ci][:],
                                         w2bs[kc][:, m2 * 128:(m2 + 1) * 128],
                                         hts[kc][:, cs],
                                         start=(kc == 0), stop=(kc == M1 - 1))
                for ci in range(nch):
                    gcs = slice(col0 + ci * 512, col0 + (ci + 1) * 512)
                    ev = oev.tile([128, 512], F32, tag="oevt2")
                    if ci % 2 == 0:
                        nc.vector.tensor_copy(ev[:], pss[ci][:])
                    else:
                        nc.scalar.copy(ev[:], pss[ci][:])
                    nc.sync.dma_start(A["part2"][m2 * 128:(m2 + 1) * 128, gcs],
                                      ev[:])


# --------------------------------------------------------------------------
# host math
# --------------------------------------------------------------------------

def _silu(x):
    return x / (1.0 + np.exp(-x))


def _resize_bilinear_8x8(a, H, W):
    """jax.image.resize(..., 'bilinear') equivalent (half-pixel centers)."""
    ih, iw = a.shape
    y = (np.arange(H, dtype=np.float64) + 0.5) * (ih / H) - 0.5
    x = (np.arange(W, dtype=np.float64) + 0.5) * (iw / W) - 0.5
    y0 = np.floor(y).astype(np.int64)
    x0 = np.floor(x).astype(np.int64)
    wy = (y - y0).astype(np.float32)
    wx = (x - x0).astype(np.float32)
    y0c = np.clip(y0, 0, ih - 1)
    y1c = np.clip(y0 + 1, 0, ih - 1)
    x0c = np.clip(x0, 0, iw - 1)
    x1c = np.clip(x0 + 1, 0, iw - 1)
    a = a.astype(np.float32)
    top = a[y0c][:, x0c] * (1 - wx)[None, :] + a[y0c][:, x1c] * wx[None, :]
    bot = a[y1c][:, x0c] * (1 - wx)[None, :] + a[y1c][:, x1c] * wx[None, :]
    return top * (1 - wy)[:, None] + bot * wy[:, None]


_PROGS = {}


def _get_progs():
    if "l1" not in _PROGS:
        _PROGS["l1"] = build_l1()
        _PROGS["l2"] = build_l2()
    return _PROGS["l1"], _PROGS["l2"]


def _run(nc, in_maps, trace=False):
    return bass_utils.run_bass_kernel_spmd(
        nc, in_maps, core_ids=list(range(NC)), trace=trace)


def kernel(txt, img, vec, rope, sol_temperature, sol_spatial,
           ada_img_w, ada_img_b, ada_img_nw, ada_txt_w, ada_txt_b, ada_txt_nw,
           txt_qkv_w, img_qkv_w, txt_out_w, img_out_w, mod_w, mod_b,
           img_n2_w, txt_n2_w, img_fc1_w, img_fc1_b, img_fc2_w, img_fc2_b,
           txt_fc1_w, txt_fc1_b, txt_fc2_w, txt_fc2_b, spatial_h, spatial_w,
           _trace=False, _timing=None):
    f = np.float32
    nc1, nc2 = _get_progs()

    x_txt = np.asarray(txt, f)[0]            # [L, HID]
    x_img = np.asarray(img, f)[0]            # [NI, HID]
    v = np.asarray(vec, f)[0]                # [HID]

    sv = _silu(v)
    e_txt = sv @ np.asarray(ada_txt_w, f) + np.asarray(ada_txt_b, f)
    e_img = sv @ np.asarray(ada_img_w, f) + np.asarray(ada_img_b, f)
    shm_t, scm_t, gm_t, shp_t, scp_t, gp_t = np.split(e_txt, 6)
    shm_i, scm_i, gm_i, shp_i, scp_i, gp_i = np.split(e_img, 6)

    a1_t = np.asarray(ada_txt_nw, f) * (1.0 + scm_t)
    a1_i = np.asarray(ada_img_nw, f) * (1.0 + scm_i)

    rs1_t = 1.0 / np.sqrt((x_txt * x_txt).mean(-1) + EPS)     # [L]
    rs1_i = 1.0 / np.sqrt((x_img * x_img).mean(-1) + EPS)     # [NI]

    # spatial modulation folded into rope cos/sin
    up = _resize_bilinear_8x8(np.asarray(sol_spatial, f)[0], 64, 32).reshape(-1)
    mod = np.exp(np.clip(up[None, :] * np.asarray(mod_w, f)[:, None]
                         + np.asarray(mod_b, f)[:, None], -2.0, 2.0))  # [NH, NI]
    fr = np.asarray(rope, f)[:, 0]                        # [NI, HD]
    cos = np.repeat(fr[:, 0::2], 2, axis=1)               # [NI, HD]
    sin = np.repeat(fr[:, 1::2], 2, axis=1)

    temp = max(float(np.asarray(sol_temperature, f).mean()), 0.1)
    s2v = SCALE / temp
    s2 = np.full((128, 1), s2v, f)
    ns2 = -s2

    permT = np.zeros((128, 128), f)
    for i in range(64):
        permT[2 * i + 1, 2 * i] = -1.0
        permT[2 * i, 2 * i + 1] = 1.0
    ident = np.eye(128, dtype=f)

    wq_t_full = np.asarray(txt_qkv_w, f)
    wq_i_full = np.asarray(img_qkv_w, f)
    wo_t_full = np.asarray(txt_out_w, f)
    wo_i_full = np.asarray(img_out_w, f)

    xt_txt = np.ascontiguousarray(x_txt.T * rs1_t[None, :]).astype(BF16)
    xt_img = np.ascontiguousarray(x_img.T * rs1_i[None, :]).astype(BF16)

    in_maps1 = []
    for c in range(NC):
        h0, h1 = 2 * c, 2 * c + 1
        cols = np.concatenate([
            np.arange(h0 * HD, (h0 + 1) * HD),
            np.arange(h1 * HD, (h1 + 1) * HD),
            HID + np.arange(h0 * HD, (h0 + 1) * HD),
            HID + np.arange(h1 * HD, (h1 + 1) * HD),
            2 * HID + np.arange(h0 * HD, (h0 + 1) * HD),
            2 * HID + np.arange(h1 * HD, (h1 + 1) * HD)])
        wq_t_raw = wq_t_full[:, cols]
        wq_i_raw = wq_i_full[:, cols]
        cm = np.stack([(cos * mod[h][:, None]).T for h in (h0, h1)]).astype(BF16)
        sm = np.stack([(sin * mod[h][:, None]).T for h in (h0, h1)]).astype(BF16)
        in_maps1.append({
            "xt_txt": xt_txt, "xt_img": xt_img,
            "wq_txt": (a1_t[:, None] * wq_t_raw).astype(BF16),
            "wq_img": (a1_i[:, None] * wq_i_raw).astype(BF16),
            "qb_txt": np.ascontiguousarray(
                (shm_t @ wq_t_raw).astype(f).reshape(6, 128).T),
            "qb_img": np.ascontiguousarray(
                (shm_i @ wq_i_raw).astype(f).reshape(6, 128).T),
            "cm": cm, "sm": sm,
            "permT": permT.astype(BF16), "ident": ident.astype(BF16),
            "identf": ident, "s2": s2, "ns2": ns2,
            "wo_txt": wo_t_full[c * 256:(c + 1) * 256, :].astype(BF16),
            "wo_img": wo_i_full[c * 256:(c + 1) * 256, :].astype(BF16),
        })

    res1 = _run(nc1, in_maps1, trace=_trace)
    if _timing is not None:
        _timing.append(("l1", res1.exec_time_ns))

    attnT = res1.results[0]["part1"].astype(f)
    for c in range(1, NC):
        attnT += res1.results[c]["part1"]

    x2_t = x_txt.T + gm_t[:, None] * attnT[:, :L]     # [HID, L]
    x2_i = x_img.T + gm_i[:, None] * attnT[:, L:]     # [HID, NI]

    rs2_t = 1.0 / np.sqrt((x2_t * x2_t).mean(0) + EPS)
    rs2_i = 1.0 / np.sqrt((x2_i * x2_i).mean(0) + EPS)
    x2t = np.concatenate([x2_t * rs2_t[None, :],
                          x2_i * rs2_i[None, :]], axis=1).astype(BF16)

    a2_t = np.asarray(txt_n2_w, f) * (1.0 + scp_t)
    a2_i = np.asarray(img_n2_w, f) * (1.0 + scp_i)
    b1f_t = shp_t @ np.asarray(txt_fc1_w, f) + np.asarray(txt_fc1_b, f)
    b1f_i = shp_i @ np.asarray(img_fc1_w, f) + np.asarray(img_fc1_b, f)

    MS = MH // NC
    in_maps2 = []
    for c in range(NC):
        csl = slice(c * MS, (c + 1) * MS)
        in_maps2.append({
            "x2t": x2t,
            "w1_txt": (a2_t[:, None]
                       * np.asarray(txt_fc1_w, f)[:, csl]).astype(BF16),
            "w1_img": (a2_i[:, None]
                       * np.asarray(img_fc1_w, f)[:, csl]).astype(BF16),
            "b1_txt": np.ascontiguousarray(b1f_t[csl].reshape(8, 128).T.astype(f)),
            "b1_img": np.ascontiguousarray(b1f_i[csl].reshape(8, 128).T.astype(f)),
            "w2_txt": np.asarray(txt_fc2_w, f)[csl, :].astype(BF16),
            "w2_img": np.asarray(img_fc2_w, f)[csl, :].astype(BF16),
        })

    res2 = _run(nc2, in_maps2, trace=_trace)
    if _timing is not None:
        _timing.append(("l2", res2.exec_time_ns))

    mlpT = res2.results[0]["part2"].astype(f)
    for c in range(1, NC):
        mlpT += res2.results[c]["part2"]

    out_t = x2_t + gp_t[:, None] * (mlpT[:, :L]
                                    + np.asarray(txt_fc2_b, f)[:, None])
    out_i = x2_i + gp_i[:, None] * (mlpT[:, L:]
                                    + np.asarray(img_fc2_b, f)[:, None])

    txt_out = np.ascontiguousarray(out_t.T).reshape(1, L, HID).astype(f)
    img_out = np.ascontiguousarray(out_i.T).reshape(1, NI, HID).astype(f)
    return txt_out, img_out
